# revision 4
# baseline (speedup 1.0000x reference)
"""ChebNet (K=2) GNN message passing on 8 TRN2 NeuronCores.

Strategy (edge sharding by destination stripe):
  - Sort edges by destination node; core c owns destinations
    [c*6272, (c+1)*6272) (N padded 50000 -> 50176 = 8*6272).
  - Host passes integer index/offset data plus the per-edge Laplacian
    weight w_e = -dinv[row]*dinv[col] (degrees are integer counts of the
    edge list, so this is cheap exact preprocessing); everything else
    (the two sparse matvecs, dense matmuls, log_softmax) runs on device.
  - Layer-1 gathers read RAW x straight from the fp32 input laid out as
    pair rows [NP/2, 128] (512B descriptors, >=512B so no sub-512B DMA
    penalty; idx = node>>1 fits int16; edges grouped per dest window by
    source parity so the matmul slices the gathered row at 64*(par)).
    No scaled bf16 table is built at all - the dinv factors ride in the
    one-hot (below), which removes the serial table-build phase.
  - Per 128-edge block the selection matrix S is built on DVE with ONE
    tensor_scalar op: S[p,q] = (iota[q] == ldst[p]) * wght[p] - per-
    partition scalar reads keep every tensor operand unit-stride so the
    op runs in the 4x DVE mode, and the edge weight (both dinv factors)
    is fused into S. PE accumulates G.T @ S into PSUM [64 feats, 128
    dests]; the drain is a plain PSUM->SBUF cast on the (otherwise
    idle) Activation engine, as are the fp32->bf16 gather conversions.
  - Layer 2: h (raw, no scaling) is computed distributed; the stripe is
    cast to fp8e4m3 and AllGathered in window-group slices (halves the
    collective bytes; rel-err ~4e-3 vs the 2e-2 gate). The fp8 table is
    quad rows [NP/4, 256] (256B descriptors), edges grouped by source
    quadrant (r2 layout = the slice concatenation order, as before);
    gathered fp8 is upcast to bf16 on Activation before PE.
  - Gathers alternate between 2 SWDGE queues (HW: one queue serializes
    desc-gen against the in-flight DMA).
"""

import os
import sys

import numpy as np

sys.path.insert(0, "/opt/trn_rl_repo")

import concourse.bacc as bacc
import concourse.bass as bass
import concourse.tile as tile
from concourse import mybir
from concourse.masks import make_identity

FP32 = mybir.dt.float32
BF16 = mybir.dt.bfloat16
FP8 = mybir.dt.float8e4
I32 = mybir.dt.int32
GMAX = int(os.environ.get("CHEB_GMAX", "64"))
NQ = int(os.environ.get("CHEB_NQ", "2"))      # SWDGE queues (1-4)
SINGLE_PACKET = os.environ.get("CHEB_SP", "0") == "1"
L2DT = os.environ.get("CHEB_L2DT", "f8")       # f8 | bf16

N = 50000
E = 800000
F = 64          # in dim
HID = 64
OUT = 40
C = 8           # cores
SN = 6272       # nodes per stripe (49 * 128)
NP = SN * C     # padded node count 50176
W = SN // 128   # 49 windows per core
NW = NP // 128  # 392 global windows
AX = mybir.AxisListType

# layer-1 table: x fp32 pair rows [NP/2, 128] (512B)
NSUB1, ES1 = 2, 128
# layer-2 table: h fp8 quad rows [NP/4, 256] (256B) or bf16 pairs
if L2DT == "f8":
    NSUB2, ES2, DT2 = 4, 256, FP8
else:
    NSUB2, ES2, DT2 = 2, 128, BF16


# ---------------------------------------------------------------- host side


# layer-2 collective window-groups: dense1+h' for each group completes
# early, so its AllGather slice overlaps the rest of layer-1 compute.
_WG = os.environ.get("CHEB_WG", "4")
if _WG == "6":
    WGRP = [(0, 8), (8, 16), (16, 24), (24, 32), (32, 40), (40, 49)]
elif _WG == "2":
    WGRP = [(0, 24), (24, 49)]
else:
    WGRP = [(0, 12), (12, 24), (24, 36), (36, 49)]


def _r2(n):
    """Layer-2 table row for node n: group-major, then rank, then
    partition-major within the stripe - exactly the concatenation order
    the per-group AllGather slices produce."""
    c, m = n // SN, n % SN
    wl, p = m // 128, m % 128
    w0s = np.array([g[0] for g in WGRP], dtype=np.int64)
    g = np.searchsorted(w0s, wl, side="right") - 1
    w0 = w0s[g]
    ng = np.array([e - s for s, e in WGRP], dtype=np.int64)[g]
    base = w0 * 128 * C
    return base + c * ng * 128 + p * ng + (wl - w0)


def _group_edges(rs, rows, wv, nsub):
    """Group the (dest-sorted) edges by (dest window, source-row residue
    mod nsub) and pad each group to whole 128-edge blocks (counts maxed
    over cores so the SPMD program is uniform).

    rows = table row id of each edge's source; gather index is
    rows >> log2(nsub) (a 256/512B multi-node row), the matmul slices
    the gathered row at offset 64*(rows % nsub).  wv = per-edge weight.
    """
    shift = nsub.bit_length() - 1
    win = rs >> 7
    sub = (rows & (nsub - 1)).astype(np.int64)
    gid = win * nsub + sub
    gorder = np.argsort(gid, kind="stable")
    rs = rs[gorder]
    rows = rows[gorder]
    wv = wv[gorder]
    gid = gid[gorder]

    gcnt = np.bincount(gid, minlength=C * W * nsub).reshape(C, W, nsub)
    nbw = (gcnt.max(axis=0) + 127) // 128          # [W, nsub]
    # guarantee each window has at least one block so PSUM is initialized
    empty = nbw.sum(axis=1) == 0
    nbw[empty, 0] = 1
    nbtot = int(nbw.sum())

    # block base per (w, sub): sub-groups contiguous within a window
    wb = np.zeros(W * nsub + 1, dtype=np.int64)
    np.cumsum(nbw.reshape(-1), out=wb[1:])
    groups = [
        tuple(
            v
            for h in range(nsub)
            for v in (int(wb[w * nsub + h]), int(nbw[w, h]))
        )
        for w in range(W)
    ]

    flat_idx = np.zeros((C, nbtot * 128), dtype=np.int16)
    ldest = np.full((C, 128, nbtot), 255.0, dtype=np.float32)
    wght = np.zeros((C, 128, nbtot), dtype=np.float32)

    starts = np.zeros(C * W * nsub + 1, dtype=np.int64)
    np.cumsum(gcnt.reshape(-1), out=starts[1:])
    for c in range(C):
        for w in range(W):
            for h in range(nsub):
                g = (c * W + w) * nsub + h
                s, e = starts[g], starts[g + 1]
                m = e - s
                if m == 0:
                    continue
                base = wb[w * nsub + h]
                ei = np.arange(m)
                b = base + (ei >> 7)
                p = ei & 127
                flat_idx[c, b * 128 + p] = (rows[s:e] >> shift).astype(
                    np.int16)
                ldest[c, p, b] = (rs[s:e] - (c * SN + w * 128)).astype(
                    np.float32)
                wght[c, p, b] = wv[s:e].astype(np.float32)

    # wrap: index i at [i % 16, i // 16], replicated down 8 partition groups
    S_tot = nbtot * 8
    idx16 = np.zeros((C, 128, S_tot), dtype=np.int16)
    wrapped = flat_idx.reshape(C, S_tot, 16).transpose(0, 2, 1)  # [C,16,S]
    for grp in range(8):
        idx16[:, grp * 16:(grp + 1) * 16, :] = wrapped

    return idx16, ldest, groups, nbtot, wght


def _pack(edge_index: np.ndarray):
    """Preprocessing: sort/partition/pad the edge list and compute the
    per-edge normalized Laplacian weight w = -dinv[row]*dinv[col]
    (degrees from the integer edge list; self-loop edges dropped).

    Two gather-index sets are emitted: layer 1 gathers raw x fp32 pair
    rows in plain node order (row = node id); layer 2 gathers from the
    AllGather output, whose row order is _r2 (group-major).
    """
    row = np.asarray(edge_index[0], dtype=np.int64)
    col = np.asarray(edge_index[1], dtype=np.int64)

    deg = np.bincount(row, minlength=NP).astype(np.float32)
    dinv = np.where(deg > 0, 1.0 / np.sqrt(np.maximum(deg, 1.0)),
                    0.0).astype(np.float32)

    order = np.argsort(row, kind="stable")
    rs = row[order]
    cs = col[order]
    keep = rs != cs
    rs = rs[keep]
    cs = cs[keep]
    wv = (-dinv[rs] * dinv[cs]).astype(np.float32)

    pk1 = _group_edges(rs, cs, wv, NSUB1)          # layer 1: row = node id
    pk2 = _group_edges(rs, _r2(cs), wv, NSUB2)     # layer 2: row = r2(node)
    return pk1, pk2


# -------------------------------------------------------------- bass program


def _build(groups1, nbtot1, groups2, nbtot2, replay=0):
    nc = bacc.Bacc(
        "TRN2",
        target_bir_lowering=False,
        debug=False,
        num_devices=C,
        num_swdge_queues=NQ,
    )

    # --- I/O
    x_full = nc.dram_tensor("x_full", [NP, F], FP32, kind="ExternalInput").ap()
    x_st = nc.dram_tensor("x_st", [SN, F], FP32, kind="ExternalInput").ap()
    idx16a = nc.dram_tensor("idx16a", [128, nbtot1 * 8],
                            mybir.dt.int16, kind="ExternalInput").ap()
    ldsta = nc.dram_tensor("ldsta", [128, nbtot1], FP32,
                           kind="ExternalInput").ap()
    wghta = nc.dram_tensor("wghta", [128, nbtot1], FP32,
                           kind="ExternalInput").ap()
    idx16b = nc.dram_tensor("idx16b", [128, nbtot2 * 8],
                            mybir.dt.int16, kind="ExternalInput").ap()
    ldstb = nc.dram_tensor("ldstb", [128, nbtot2], FP32,
                           kind="ExternalInput").ap()
    wghtb = nc.dram_tensor("wghtb", [128, nbtot2], FP32,
                           kind="ExternalInput").ap()
    w01 = nc.dram_tensor("w01", [F, HID], FP32, kind="ExternalInput").ap()
    w11 = nc.dram_tensor("w11", [F, HID], FP32, kind="ExternalInput").ap()
    b1 = nc.dram_tensor("b1", [HID, 1], FP32, kind="ExternalInput").ap()
    w02 = nc.dram_tensor("w02", [HID, OUT], FP32, kind="ExternalInput").ap()
    w12 = nc.dram_tensor("w12", [HID, OUT], FP32, kind="ExternalInput").ap()
    b2 = nc.dram_tensor("b2", [OUT, 1], FP32, kind="ExternalInput").ap()
    out = nc.dram_tensor("out", [SN, OUT], FP32, kind="ExternalOutput").ap()

    # --- internal DRAM (layer-2 gather table + collective bounce)
    hp_b = nc.dram_tensor("hp_b", [SN, F], DT2).ap()
    hp_full = nc.dram_tensor("hp_full", [NP, F], DT2,
                             addr_space="Shared").ap()

    def wtot(g):
        return sum(g[2 * h + 1] for h in range(len(g) // 2))

    nwmax = max(wtot(g) for g in groups1 + groups2)
    rg = [list(range(C))]

    with tile.TileContext(nc) as tc:
        with (
            tc.tile_pool(name="const", bufs=1) as cpool,
            tc.tile_pool(name="big", bufs=1) as bpool,
            tc.tile_pool(name="work", bufs=2) as wpool,
            tc.tile_pool(name="gpool", bufs=int(os.environ.get("CHEB_GB", "3"))) as gpool,
            tc.tile_pool(name="spool", bufs=int(os.environ.get("CHEB_SB", "3"))) as spool,
            tc.tile_pool(name="psw", bufs=4, space="PSUM") as psw,
            tc.tile_pool(name="psd", bufs=2, space="PSUM") as psd,
            tc.tile_pool(name="pst", bufs=2, space="PSUM") as pst,
        ):
            def body(variant="full"):
                front_on = variant in ("full", "front", "nogather",
                                       "nocoll")
                gather_on = variant in ("full", "gatheronly", "nocoll")
                compute_on = variant in ("full", "nogather", "nocoll")
                coll_on = variant in ("full", "nogather")

                nbmx = max(nbtot1, nbtot2)

                def load_edges(idx_t, ldst_t, wght_t, nbtot):
                    idx_s = cpool.tile([128, nbmx * 8], mybir.dt.int16,
                                       tag="ei")
                    nc.sync.dma_start(out=idx_s[:, :nbtot * 8], in_=idx_t)
                    ldst_s = cpool.tile([128, nbmx], FP32, tag="el")
                    nc.sync.dma_start(out=ldst_s[:, :nbtot], in_=ldst_t)
                    wght_s = cpool.tile([128, nbmx], FP32, tag="ew")
                    nc.sync.dma_start(out=wght_s[:, :nbtot], in_=wght_t)
                    return idx_s, ldst_s, wght_s

                idx_a, ldst_a, wght_a = load_edges(idx16a, ldsta, wghta,
                                                   nbtot1)

                if front_on:
                    ident = cpool.tile([128, 128], FP32, tag="ident")
                    make_identity(nc, ident[:])
                    ident_b = cpool.tile([128, 128], BF16, tag="ident_b")
                    nc.vector.tensor_copy(ident_b[:], ident[:])

                    iota_i = cpool.tile([128, 128], I32, tag="iota_i")
                    nc.gpsimd.iota(iota_i[:], pattern=[[1, 128]], base=0,
                                   channel_multiplier=0)
                    iota_f = cpool.tile([128, 128], BF16, tag="iota_f")
                    nc.vector.tensor_copy(iota_f[:], iota_i[:])

                    wts = {}
                    for nm, src, shp in (("w01", w01, [F, HID]),
                                         ("w11", w11, [F, HID]),
                                         ("w02", w02, [HID, OUT]),
                                         ("w12", w12, [HID, OUT])):
                        f32 = cpool.tile(shp, FP32, tag=nm + "_f")
                        nc.sync.dma_start(out=f32[:], in_=src)
                        bft = cpool.tile(shp, BF16, tag=nm)
                        nc.vector.tensor_copy(bft[:], f32[:])
                        wts[nm] = bft
                    b1_s = cpool.tile([HID, 1], FP32, tag="b1")
                    b2_s = cpool.tile([OUT, 1], FP32, tag="b2")
                    nc.sync.dma_start(out=b1_s[:], in_=b1)
                    nc.sync.dma_start(out=b2_s[:], in_=b2)

                if variant == "front":
                    return

                if compute_on:
                    # ---- own stripe: xT (bf16) for dense layer 1
                    x_sb = bpool.tile([128, W * F], FP32, tag="x_sb")
                    nc.sync.dma_start(
                        out=x_sb[:].rearrange("p (w f) -> p w f", w=W),
                        in_=x_st.rearrange("(w p) f -> p w f", p=128))
                    xT = bpool.tile([F, SN], BF16, tag="xT")
                    for w in range(W):
                        ps_t = pst.tile([F, 128], FP32, tag="pt")
                        nc.tensor.transpose(out=ps_t[:],
                                            in_=x_sb[:, w * F:(w + 1) * F],
                                            identity=ident[:])
                        nc.vector.tensor_copy(xT[:, w * 128:(w + 1) * 128],
                                              ps_t[:])

                # ---- sparse matvec: multi-node rows gathered from tp;
                # S = onehot(ldst) * wght built per block on DVE; PE
                # accumulates G.T @ S; Activation drains PSUM -> dst.
                gctr = [0]

                def matvec(tp, es, nsub, gdt, dst, idx_s, ldst_s, wght_s,
                           groups, w0=0, w1=W, do_gather=True,
                           do_compute=True):
                    for w in range(w0, w1):
                        g_w = groups[w]
                        subs = [(g_w[2 * h], g_w[2 * h + 1])
                                for h in range(nsub)]
                        base = subs[0][0]
                        ntot = wtot(g_w)
                        if do_compute:
                            pw = psw.tile([F, 128], FP32, tag="pw",
                                          name="pw")
                            s = spool.tile([128, nwmax * 128], BF16,
                                           tag="S", name="s")
                            for b in range(ntot):
                                nc.vector.tensor_scalar(
                                    out=s[:, b * 128:(b + 1) * 128],
                                    in0=iota_f[:],
                                    scalar1=ldst_s[:, base + b:base + b + 1],
                                    scalar2=wght_s[:, base + b:base + b + 1],
                                    op0=mybir.AluOpType.is_equal,
                                    op1=mybir.AluOpType.mult)
                        g = gpool.tile([128, nwmax * es], gdt,
                                       tag="G%d" % es)
                        gb = gpool.tile([128, nwmax * F], BF16, tag="Gb")
                        for sub in range(0, ntot, GMAX):
                            n = min(GMAX, ntot - sub)
                            if do_gather:
                                nc.gpsimd.dma_gather(
                                    out_ap=g[:, sub * es:(sub + n) * es
                                             ].rearrange(
                                        "p (b f) -> p b f", b=n),
                                    in_ap=tp,
                                    idxs_ap=idx_s[:, (base + sub) * 8:
                                                  (base + sub + n) * 8],
                                    num_idxs=n * 128,
                                    num_idxs_reg=n * 128,
                                    elem_size=es,
                                    single_packet=SINGLE_PACKET,
                                    queue_num=gctr[0] % NQ,
                                )
                                gctr[0] += 1
                        if not do_compute:
                            continue
                        # upcast each sub-group's slice to bf16 (fused
                        # parity/quadrant select) on Activation
                        g3 = g[:, :ntot * es].rearrange("p (b f) -> p b f",
                                                        b=ntot)
                        gb3 = gb[:, :ntot * F].rearrange("p (b f) -> p b f",
                                                         b=ntot)
                        for h, (hb, hn) in enumerate(subs):
                            if hn == 0:
                                continue
                            o = hb - base
                            if do_gather:
                                nc.scalar.copy(
                                    out=gb3[:, o:o + hn, :],
                                    in_=g3[:, o:o + hn,
                                           h * F:(h + 1) * F])
                            else:
                                # replay variant: source the iota const
                                # (same shape/dtype -> same Act timing)
                                io_ap = iota_f[:, 0:F]
                                nc.scalar.copy(
                                    out=gb3[:, o:o + hn, :],
                                    in_=bass.AP(io_ap.tensor, io_ap.offset,
                                                [io_ap.ap[0], [0, hn],
                                                 io_ap.ap[1]]))
                        for b in range(ntot):
                            nc.tensor.matmul(
                                out=pw[:],
                                lhsT=gb[:, b * F:(b + 1) * F],
                                rhs=s[:, b * 128:(b + 1) * 128],
                                start=(b == 0),
                                stop=(b == ntot - 1))
                        nc.scalar.copy(out=dst[:, w * 128:(w + 1) * 128],
                                       in_=pw[:])

                tp1 = bass.AP(x_full.tensor, x_full.offset,
                              [[ES1, NP // NSUB1], [1, ES1]])
                tp2 = bass.AP(hp_full.tensor, hp_full.offset,
                              [[ES2, NP // NSUB2], [1, ES2]])

                txT = bpool.tile([F, SN], BF16, tag="txT")
                hT = bpool.tile([HID, SN], BF16, tag="hT")
                hp_sb = bpool.tile([128, W * F], DT2, tag="hp_sb")
                tx2T = bpool.tile([F, SN], BF16, tag="tx2T")
                out_sb = bpool.tile([128, W * OUT], FP32, tag="out_sb")
                nchunk = (SN + 511) // 512

                # ---- dense layer 1: hT = relu(W01.T @ xT + W11.T @ txT + b1)
                def dense1(c0, c1):
                    for lo in range(c0, c1, 512):
                        m = min(512, c1 - lo)
                        pd = psd.tile([HID, 512], FP32, tag="pd")
                        nc.tensor.matmul(out=pd[:, :m], lhsT=wts["w01"][:],
                                         rhs=xT[:, lo:lo + m], start=True,
                                         stop=False)
                        nc.tensor.matmul(out=pd[:, :m], lhsT=wts["w11"][:],
                                         rhs=txT[:, lo:lo + m], start=False,
                                         stop=True)
                        nc.scalar.activation(
                            out=hT[:, lo:lo + m], in_=pd[:, :m],
                            func=mybir.ActivationFunctionType.Relu,
                            bias=b1_s[:], scale=1.0)

                # ---- h stripe -> collective dtype (node-major)
                def hp_compute(w0, w1):
                    for w in range(w0, w1):
                        ps_h = pst.tile([128, F], BF16, tag="pt")
                        nc.tensor.transpose(out=ps_h[:],
                                            in_=hT[:, w * 128:(w + 1) * 128],
                                            identity=ident_b[:F, :F])
                        nc.vector.tensor_copy(
                            hp_sb[:, w * F:(w + 1) * F], ps_h[:])

                # ---- layer 1 + the h AllGather, pipelined per window
                # group: each group's collective slice fires as soon as its
                # windows' dense1 + cast are done, overlapping the rest of
                # layer-1 compute. hp_b row = W0*128 + p*ng + (wl-W0), the
                # concat order the per-group AllGather produces (= _r2).
                for (gw0, gw1) in WGRP:
                    ng = gw1 - gw0
                    matvec(tp1, ES1, NSUB1, FP32, txT, idx_a, ldst_a,
                           wght_a, groups1, gw0, gw1,
                           do_gather=gather_on, do_compute=compute_on)
                    if compute_on:
                        dense1(gw0 * 128, gw1 * 128)
                        hp_compute(gw0, gw1)
                        nc.sync.dma_start(
                            out=hp_b[gw0 * 128:gw1 * 128].rearrange(
                                "(p j) f -> p j f", p=128),
                            in_=hp_sb[:, gw0 * F:gw1 * F].rearrange(
                                "p (j f) -> p j f", j=ng))
                    if coll_on:
                        nc.gpsimd.collective_compute(
                            "AllGather",
                            mybir.AluOpType.bypass,
                            ins=[hp_b[gw0 * 128:gw1 * 128]],
                            outs=[hp_full[gw0 * 128 * C:gw1 * 128 * C]],
                            replica_groups=rg,
                        )

                # ---- sparse matvec 2 (gathers straight from the AllGather
                # output - row order is _r2, no post-collective shuffle)
                idx_b, ldst_b, wght_b = load_edges(idx16b, ldstb, wghtb,
                                                   nbtot2)
                matvec(tp2, ES2, NSUB2, DT2, tx2T, idx_b, ldst_b, wght_b,
                       groups2, do_gather=gather_on, do_compute=compute_on)
                if not compute_on:
                    return

                # ---- dense layer 2 + bias + transpose + log_softmax
                def dense2_softmax():
                    o_all = bpool.tile([128, W * OUT], FP32, tag="o_all")
                    for i in range(nchunk):
                        lo = i * 512
                        m = min(512, SN - lo)
                        pd = psd.tile([OUT, 512], FP32, tag="pd")
                        nc.tensor.matmul(out=pd[:, :m], lhsT=wts["w02"][:],
                                         rhs=hT[:, lo:lo + m], start=True,
                                         stop=False)
                        nc.tensor.matmul(out=pd[:, :m], lhsT=wts["w12"][:],
                                         rhs=tx2T[:, lo:lo + m], start=False,
                                         stop=True)
                        ob = wpool.tile([OUT, 512], FP32, tag="ob")
                        nc.vector.tensor_scalar(
                            out=ob[:, :m], in0=pd[:, :m], scalar1=b2_s[:],
                            scalar2=None, op0=mybir.AluOpType.add)
                        for j in range(m // 128):
                            w = i * 4 + j
                            ps_o = pst.tile([128, OUT], FP32, tag="pt")
                            nc.tensor.transpose(
                                out=ps_o[:], in_=ob[:, j * 128:(j + 1) * 128],
                                identity=ident[:OUT, :OUT])
                            nc.vector.tensor_copy(
                                o_all[:, w * OUT:(w + 1) * OUT], ps_o[:])
                    e_all = bpool.tile([128, W * OUT], FP32, tag="e_all")
                    nc.scalar.activation(out=e_all[:], in_=o_all[:],
                                         func=mybir.ActivationFunctionType.Exp)
                    ssum = wpool.tile([128, W], FP32, tag="ssum")
                    nc.vector.tensor_reduce(
                        out=ssum[:],
                        in_=e_all[:].rearrange("p (w q) -> p w q", w=W),
                        axis=AX.X, op=mybir.AluOpType.add)
                    lns = wpool.tile([128, W], FP32, tag="lns")
                    nc.scalar.activation(out=lns[:], in_=ssum[:],
                                         func=mybir.ActivationFunctionType.Ln)
                    ln_ap = lns[:]
                    nc.vector.tensor_tensor(
                        out=out_sb[:].rearrange("p (w q) -> p w q", w=W),
                        in0=o_all[:].rearrange("p (w q) -> p w q", w=W),
                        in1=bass.AP(ln_ap.tensor, ln_ap.offset,
                                    [ln_ap.ap[0], ln_ap.ap[1], [0, OUT]]),
                        op=mybir.AluOpType.subtract)

                dense2_softmax()
                nc.sync.dma_start(
                    out=out.rearrange("(w p) f -> p w f", p=128),
                    in_=out_sb[:].rearrange("p (w f) -> p w f", w=W))

            rm = os.environ.get("CHEB_RM", "full")
            body("full")
            for _ in range(replay):
                body(rm)

    nc.compile()
    return nc


# ------------------------------------------------------------------- driver

_CACHE = {}


def _get_program_and_maps(x, edge_index, W0_1, W1_1, b1, W0_2, W1_2, b2):
    pk1, pk2 = _pack(np.asarray(edge_index))
    idx16a, ldesta, groups1, nbtot1, wghta = pk1
    idx16b, ldestb, groups2, nbtot2, wghtb = pk2

    x_pad = np.zeros((NP, F), dtype=np.float32)
    x_pad[:N] = np.asarray(x, dtype=np.float32)

    key = tuple(v for g in groups1 + groups2 for v in g)
    if key not in _CACHE:
        _CACHE[key] = _build(groups1, nbtot1, groups2, nbtot2)
    nc = _CACHE[key]

    shared = {
        "x_full": x_pad,
        "w01": np.asarray(W0_1, np.float32),
        "w11": np.asarray(W1_1, np.float32),
        "b1": np.asarray(b1, np.float32).reshape(HID, 1),
        "w02": np.asarray(W0_2, np.float32),
        "w12": np.asarray(W1_2, np.float32),
        "b2": np.asarray(b2, np.float32).reshape(OUT, 1),
    }
    in_maps = []
    for c in range(C):
        m = dict(shared)
        m["x_st"] = np.ascontiguousarray(x_pad[c * SN:(c + 1) * SN])
        m["idx16a"] = np.ascontiguousarray(idx16a[c])
        m["ldsta"] = np.ascontiguousarray(ldesta[c])
        m["wghta"] = np.ascontiguousarray(wghta[c])
        m["idx16b"] = np.ascontiguousarray(idx16b[c])
        m["ldstb"] = np.ascontiguousarray(ldestb[c])
        m["wghtb"] = np.ascontiguousarray(wghtb[c])
        in_maps.append(m)
    return nc, in_maps


def kernel(x, edge_index, W0_1, W1_1, b1, W0_2, W1_2, b2, **kw):
    nc, in_maps = _get_program_and_maps(
        x, edge_index, W0_1, W1_1, b1, W0_2, W1_2, b2)

    from concourse.bass_utils import run_bass_kernel_spmd

    res = run_bass_kernel_spmd(nc, in_maps, core_ids=list(range(C)))
    outs = [res.results[c]["out"] for c in range(C)]
    full = np.concatenate(outs, axis=0)[:N]
    return full.astype(np.float32)


# revision 21
# speedup vs baseline: 1.1747x; 1.1747x over previous
"""ChebNet (K=2) GNN message passing on 8 TRN2 NeuronCores.

Strategy (edge sharding by destination stripe):
  - Sort edges by destination node; core c owns destinations
    [c*6272, (c+1)*6272) (N padded 50000 -> 50176 = 8*6272).
  - Host passes integer index/offset data plus the per-edge Laplacian
    weight w_e = -dinv[row]*dinv[col] (degrees are integer counts of the
    edge list, so this is cheap exact preprocessing); everything else
    (the two sparse matvecs, dense matmuls, log_softmax) runs on device.
  - Layer-1 gathers read RAW x straight from the fp32 input laid out as
    pair rows [NP/2, 128] (512B descriptors, >=512B so no sub-512B DMA
    penalty; idx = node>>1 fits int16; edges grouped per dest window by
    source parity so the matmul slices the gathered row at 64*(par)).
    No scaled bf16 table is built at all - the dinv factors ride in the
    one-hot (below), which removes the serial table-build phase.
  - Per 128-edge block the selection matrix S is built on DVE with ONE
    tensor_scalar op: S[p,q] = (iota[q] == ldst[p]) * wght[p] - per-
    partition scalar reads keep every tensor operand unit-stride so the
    op runs in the 4x DVE mode, and the edge weight (both dinv factors)
    is fused into S. PE accumulates G.T @ S into PSUM [64 feats, 128
    dests]; the drain is a plain PSUM->SBUF cast on the (otherwise
    idle) Activation engine, as are the fp32->bf16 gather conversions.
  - Layer 2: h (raw, no scaling) is computed distributed; the stripe is
    cast to fp8e4m3 and AllGathered in window-group slices (halves the
    collective bytes; rel-err ~4e-3 vs the 2e-2 gate). The fp8 table is
    quad rows [NP/4, 256] (256B descriptors), edges grouped by source
    quadrant (r2 layout = the slice concatenation order, as before);
    gathered fp8 is upcast to bf16 on Activation before PE.
  - Gathers alternate between 2 SWDGE queues (HW: one queue serializes
    desc-gen against the in-flight DMA).
"""

import os
import sys

import numpy as np

sys.path.insert(0, "/opt/trn_rl_repo")

import concourse.bacc as bacc
import concourse.bass as bass
import concourse.tile as tile
from concourse import mybir
from concourse.masks import make_identity

FP32 = mybir.dt.float32
BF16 = mybir.dt.bfloat16
FP8 = mybir.dt.float8e4
I32 = mybir.dt.int32
GMAX = int(os.environ.get("CHEB_GMAX", "64"))
NQ = int(os.environ.get("CHEB_NQ", "2"))      # SWDGE queues (1-4)
SINGLE_PACKET = os.environ.get("CHEB_SP", "0") == "1"
L2DT = os.environ.get("CHEB_L2DT", "f8")       # f8 | bf16

N = 50000
E = 800000
F = 64          # in dim
HID = 64
OUT = 40
C = 8           # cores
SN = 6272       # nodes per stripe (49 * 128)
NP = SN * C     # padded node count 50176
W = SN // 128   # 49 windows per core
NW = NP // 128  # 392 global windows
AX = mybir.AxisListType

# layer-1 table: x fp32 pair rows [NP/2, 128] (512B)
NSUB1, ES1 = 2, 128
# layer-2 table: h fp8 quad rows [NP/4, 256] (256B) or bf16 pairs
if L2DT == "f8":
    NSUB2, ES2, DT2 = 4, 256, FP8
else:
    NSUB2, ES2, DT2 = 2, 128, BF16


# ---------------------------------------------------------------- host side


# layer-2 collective window-groups: dense1+h' for each group completes
# early, so its AllGather slice overlaps the rest of layer-1 compute.
_WG = os.environ.get("CHEB_WG", "5u")
if _WG == "6":
    WGRP = [(0, 8), (8, 16), (16, 24), (24, 32), (32, 40), (40, 49)]
elif _WG == "2":
    WGRP = [(0, 24), (24, 49)]
elif _WG == "4":
    WGRP = [(0, 12), (12, 24), (24, 36), (36, 49)]
else:
    WGRP = [(0, 9), (9, 19), (19, 30), (30, 40), (40, 49)]


def _r2(n):
    """Layer-2 table row for node n: group-major, then rank, then
    partition-major within the stripe - exactly the concatenation order
    the per-group AllGather slices produce."""
    c, m = n // SN, n % SN
    wl, p = m // 128, m % 128
    w0s = np.array([g[0] for g in WGRP], dtype=np.int64)
    g = np.searchsorted(w0s, wl, side="right") - 1
    w0 = w0s[g]
    ng = np.array([e - s for s, e in WGRP], dtype=np.int64)[g]
    base = w0 * 128 * C
    return base + c * ng * 128 + p * ng + (wl - w0)


def _group_edges(rs, rows, wv, nsub):
    """Group the (dest-sorted) edges by (dest window, source-row residue
    mod nsub), packing the sub-groups back-to-back within each window:
    only the WINDOW total is padded to whole 128-edge blocks (counts
    maxed over cores so the SPMD program is uniform), and the gather for
    a window fetches only the true (maxed) edge count - the tail of its
    last block is never fetched.

    rows = table row id of each edge's source; gather index is
    rows >> log2(nsub) (a 256/512B multi-node row).  A sub-group's span
    may straddle block boundaries, so the unit of S-build/matmul is a
    (block, sub) PAIR: pair j of a window owns ldst/wght column j and
    the matmul slices block b_j's gathered rows at offset 64*h_j.

    Returns (idx16, ldest, groups, nbtot, wght, npairs) with
      groups[w] = (block_base, nblocks, num_idxs_true, pair_col_base,
                   [(b_local, h, sub_nblocks...)...]) encoded as
      (bb, nb, ni, pc, [(h, tb0, nt), ...]) where sub h's nt pairs map
      1:1 onto blocks tb0..tb0+nt-1 and columns pc+k.
    """
    shift = nsub.bit_length() - 1
    win = rs >> 7
    sub = (rows & (nsub - 1)).astype(np.int64)
    gid = win * nsub + sub
    gorder = np.argsort(gid, kind="stable")
    rs = rs[gorder]
    rows = rows[gorder]
    wv = wv[gorder]
    gid = gid[gorder]

    gcnt = np.bincount(gid, minlength=C * W * nsub).reshape(C, W, nsub)
    m_wh = gcnt.max(axis=0)                        # [W, nsub]
    # guarantee each window has at least one slot so PSUM is initialized
    empty = m_wh.sum(axis=1) == 0
    m_wh[empty, 0] = 1
    q_wh = np.cumsum(m_wh, axis=1) - m_wh          # start pos of sub h
    M_w = m_wh.sum(axis=1)                         # true idx count
    B_w = (M_w + 127) // 128                       # blocks per window
    bb_w = np.concatenate([[0], np.cumsum(B_w)])
    nbtot = int(bb_w[-1])

    groups = []
    pc = 0
    for w in range(W):
        subs = []
        for h in range(nsub):
            m = int(m_wh[w, h])
            if m == 0:
                continue
            q0 = int(q_wh[w, h])
            tb0 = q0 >> 7
            nt = ((q0 + m - 1) >> 7) - tb0 + 1
            subs.append((h, tb0, nt))
        groups.append((int(bb_w[w]), int(B_w[w]), int(M_w[w]), pc,
                       tuple(subs)))
        pc += sum(s[2] for s in subs)
    npairs = pc

    flat_idx = np.zeros((C, nbtot * 128), dtype=np.int16)
    ldest = np.full((C, 128, npairs), 255.0, dtype=np.float32)
    wght = np.zeros((C, 128, npairs), dtype=np.float32)

    starts = np.zeros(C * W * nsub + 1, dtype=np.int64)
    np.cumsum(gcnt.reshape(-1), out=starts[1:])
    for c in range(C):
        for w in range(W):
            bb, nb, ni, pcw, subs = groups[w]
            coff = pcw
            for (h, tb0, nt) in subs:
                g = (c * W + w) * nsub + h
                s, e = starts[g], starts[g + 1]
                m = e - s
                if m:
                    q0 = int(q_wh[w, h])
                    pos = q0 + np.arange(m)            # window slot
                    b = pos >> 7                       # local block
                    p = pos & 127
                    flat_idx[c, (bb + b) * 128 + p] = (
                        rows[s:e] >> shift).astype(np.int16)
                    # pair column = coff + (b - tb0)
                    ldest[c, p, coff + b - tb0] = (
                        rs[s:e] - (c * SN + w * 128)).astype(np.float32)
                    wght[c, p, coff + b - tb0] = wv[s:e].astype(np.float32)
                coff += nt

    # wrap: index i at [i % 16, i // 16], replicated down 8 partition groups
    S_tot = nbtot * 8
    idx16 = np.zeros((C, 128, S_tot), dtype=np.int16)
    wrapped = flat_idx.reshape(C, S_tot, 16).transpose(0, 2, 1)  # [C,16,S]
    for grp in range(8):
        idx16[:, grp * 16:(grp + 1) * 16, :] = wrapped

    return idx16, ldest, groups, nbtot, wght, npairs


# layer-2 source split: sources whose r2 row is below SPLIT_R2 are
# covered by the first SPLIT_NS collective slices, so their gathers can
# start while the remaining slices are still in flight (pass A); the
# rest go in pass B whose drain adds the pass-A partial.
SPLIT_NS = int(os.environ.get("CHEB_SPLIT", "3"))
SPLIT_W = WGRP[SPLIT_NS - 1][1] if 0 < SPLIT_NS < len(WGRP) else 0
SPLIT_R2 = SPLIT_W * 128 * C


def _pack(edge_index: np.ndarray):
    """Preprocessing: sort/partition/pad the edge list and compute the
    per-edge normalized Laplacian weight w = -dinv[row]*dinv[col]
    (degrees from the integer edge list; self-loop edges dropped).

    Three gather-index sets are emitted: layer 1 gathers raw x fp32 pair
    rows in plain node order (row = node id); layer 2 gathers from the
    AllGather output, whose row order is _r2 (group-major), split into
    pass A (r2 < SPLIT_R2) and pass B (the rest, rows rebased so the
    gather indexes a table anchored at SPLIT_R2).
    """
    row = np.asarray(edge_index[0], dtype=np.int64)
    col = np.asarray(edge_index[1], dtype=np.int64)

    deg = np.bincount(row, minlength=NP).astype(np.float32)
    dinv = np.where(deg > 0, 1.0 / np.sqrt(np.maximum(deg, 1.0)),
                    0.0).astype(np.float32)

    order = np.argsort(row, kind="stable")
    rs = row[order]
    cs = col[order]
    keep = rs != cs
    rs = rs[keep]
    cs = cs[keep]
    wv = (-dinv[rs] * dinv[cs]).astype(np.float32)

    pk1 = _group_edges(rs, cs, wv, NSUB1)          # layer 1: row = node id
    r2c = _r2(cs)
    ma = r2c < SPLIT_R2
    mb = ~ma
    pk2a = _group_edges(rs[ma], r2c[ma], wv[ma], NSUB2)
    pk2b = _group_edges(rs[mb], r2c[mb] - SPLIT_R2, wv[mb], NSUB2)
    return pk1, pk2a, pk2b


# -------------------------------------------------------------- bass program


def _build(groups1, nbtot1, groups2, nbtot2, replay=0):
    nc = bacc.Bacc(
        "TRN2",
        target_bir_lowering=False,
        debug=False,
        num_devices=C,
        num_swdge_queues=NQ,
    )

    # --- I/O
    x_full = nc.dram_tensor("x_full", [NP, F], FP32, kind="ExternalInput").ap()
    x_st = nc.dram_tensor("x_st", [SN, F], FP32, kind="ExternalInput").ap()
    def wpairs0(g):
        return sum(s[2] for s in g[4])

    np1 = groups1[-1][3] + wpairs0(groups1[-1])
    np2 = groups2[-1][3] + wpairs0(groups2[-1])
    idx16a = nc.dram_tensor("idx16a", [128, nbtot1 * 8],
                            mybir.dt.int16, kind="ExternalInput").ap()
    ldsta = nc.dram_tensor("ldsta", [128, np1], FP32,
                           kind="ExternalInput").ap()
    wghta = nc.dram_tensor("wghta", [128, np1], FP32,
                           kind="ExternalInput").ap()
    idx16b = nc.dram_tensor("idx16b", [128, nbtot2 * 8],
                            mybir.dt.int16, kind="ExternalInput").ap()
    ldstb = nc.dram_tensor("ldstb", [128, np2], FP32,
                           kind="ExternalInput").ap()
    wghtb = nc.dram_tensor("wghtb", [128, np2], FP32,
                           kind="ExternalInput").ap()
    w01 = nc.dram_tensor("w01", [F, HID], FP32, kind="ExternalInput").ap()
    w11 = nc.dram_tensor("w11", [F, HID], FP32, kind="ExternalInput").ap()
    b1 = nc.dram_tensor("b1", [HID, 1], FP32, kind="ExternalInput").ap()
    w02 = nc.dram_tensor("w02", [HID, OUT], FP32, kind="ExternalInput").ap()
    w12 = nc.dram_tensor("w12", [HID, OUT], FP32, kind="ExternalInput").ap()
    b2 = nc.dram_tensor("b2", [OUT, 1], FP32, kind="ExternalInput").ap()
    out = nc.dram_tensor("out", [SN, OUT], FP32, kind="ExternalOutput").ap()

    # --- internal DRAM (layer-2 gather table + collective bounce)
    hp_b = nc.dram_tensor("hp_b", [SN, F], DT2).ap()
    hp_full = nc.dram_tensor("hp_full", [NP, F], DT2,
                             addr_space="Shared").ap()

    def wpairs(g):
        return sum(s[2] for s in g[4])

    bmax = max(g[1] for g in groups1 + groups2)
    pmax = max(wpairs(g) for g in groups1 + groups2)
    npairs1 = groups1[-1][3] + wpairs(groups1[-1])
    npairs2 = groups2[-1][3] + wpairs(groups2[-1])
    rg = [list(range(C))]

    with tile.TileContext(nc) as tc:
        with (
            tc.tile_pool(name="const", bufs=1) as cpool,
            tc.tile_pool(name="big", bufs=1) as bpool,
            tc.tile_pool(name="work", bufs=2) as wpool,
            tc.tile_pool(name="gpool", bufs=int(os.environ.get("CHEB_GB", "3"))) as gpool,
            tc.tile_pool(name="spool", bufs=int(os.environ.get("CHEB_SB", "3"))) as spool,
            tc.tile_pool(name="psw", bufs=4, space="PSUM") as psw,
            tc.tile_pool(name="psd", bufs=2, space="PSUM") as psd,
            tc.tile_pool(name="pst", bufs=2, space="PSUM") as pst,
        ):
            def body(variant="full"):
                front_on = variant in ("full", "front", "nogather",
                                       "nocoll")
                gather_on = variant in ("full", "gatheronly", "nocoll")
                compute_on = variant in ("full", "nogather", "nocoll")
                coll_on = variant in ("full", "nogather")

                nbmx = max(nbtot1, nbtot2)
                npmx = max(npairs1, npairs2)

                def load_edges(idx_t, ldst_t, wght_t, nbtot, npairs):
                    idx_s = cpool.tile([128, nbmx * 8], mybir.dt.int16,
                                       tag="ei")
                    nc.sync.dma_start(out=idx_s[:, :nbtot * 8], in_=idx_t)
                    ldst_s = cpool.tile([128, npmx], FP32, tag="el")
                    nc.sync.dma_start(out=ldst_s[:, :npairs], in_=ldst_t)
                    wght_s = cpool.tile([128, npmx], FP32, tag="ew")
                    nc.sync.dma_start(out=wght_s[:, :npairs], in_=wght_t)
                    return idx_s, ldst_s, wght_s

                idx_a, ldst_a, wght_a = load_edges(idx16a, ldsta, wghta,
                                                   nbtot1, npairs1)

                if front_on:
                    # zero the gather-destination pool buffers once so the
                    # unfetched tail of a truncated gather is never a read
                    # of uninitialized SBUF (NaN bits); afterwards those
                    # bytes always hold zeros or stale gathered rows.
                    for _ in range(int(os.environ.get("CHEB_GB", "3"))):
                        for tg, esz, dt in (("Ga", ES1, FP32),
                                            ("Gl2", ES2, DT2)):
                            gz = gpool.tile([128, bmax * esz], dt, tag=tg)
                            nc.vector.memset(gz[:], 0)

                    ident = cpool.tile([128, 128], FP32, tag="ident")
                    make_identity(nc, ident[:])
                    ident_b = cpool.tile([128, 128], BF16, tag="ident_b")
                    nc.vector.tensor_copy(ident_b[:], ident[:])

                    iota_i = cpool.tile([128, 128], I32, tag="iota_i")
                    nc.gpsimd.iota(iota_i[:], pattern=[[1, 128]], base=0,
                                   channel_multiplier=0)
                    iota_f = cpool.tile([128, 128], BF16, tag="iota_f")
                    nc.vector.tensor_copy(iota_f[:], iota_i[:])

                    wts = {}
                    for nm, src, shp in (("w01", w01, [F, HID]),
                                         ("w11", w11, [F, HID]),
                                         ("w02", w02, [HID, OUT]),
                                         ("w12", w12, [HID, OUT])):
                        f32 = cpool.tile(shp, FP32, tag=nm + "_f")
                        nc.sync.dma_start(out=f32[:], in_=src)
                        bft = cpool.tile(shp, BF16, tag=nm)
                        nc.vector.tensor_copy(bft[:], f32[:])
                        wts[nm] = bft
                    b1_s = cpool.tile([HID, 1], FP32, tag="b1")
                    b2_s = cpool.tile([OUT, 1], FP32, tag="b2")
                    nc.sync.dma_start(out=b1_s[:], in_=b1)
                    nc.sync.dma_start(out=b2_s[:], in_=b2)

                if variant == "front":
                    return

                if compute_on:
                    # ---- own stripe: xT (bf16) for dense layer 1
                    x_sb = bpool.tile([128, W * F], FP32, tag="x_sb")
                    nc.sync.dma_start(
                        out=x_sb[:].rearrange("p (w f) -> p w f", w=W),
                        in_=x_st.rearrange("(w p) f -> p w f", p=128))
                    xT = bpool.tile([F, SN], BF16, tag="xT")
                    for w in range(W):
                        ps_t = pst.tile([F, 128], FP32, tag="pt")
                        nc.tensor.transpose(out=ps_t[:],
                                            in_=x_sb[:, w * F:(w + 1) * F],
                                            identity=ident[:])
                        nc.vector.tensor_copy(xT[:, w * 128:(w + 1) * 128],
                                              ps_t[:])

                # ---- sparse matvec: multi-node rows gathered from tp;
                # S = onehot(ldst) * wght built per block on DVE; PE
                # accumulates G.T @ S; Activation drains PSUM -> dst.
                gctr = [0]

                def matvec(tp, es, nsub, gdt, gtag, dst, idx_s, ldst_s,
                           wght_s, groups, w0=0, w1=W, do_gather=True,
                           do_compute=True):
                    for w in range(w0, w1):
                        bb, nb, ni, pcw, subs = groups[w]
                        npw = sum(s_[2] for s_ in subs)
                        if do_compute:
                            pw = psw.tile([F, 128], FP32, tag="pw",
                                          name="pw")
                            s = spool.tile([128, pmax * 128], BF16,
                                           tag="S", name="s")
                            for j in range(npw):
                                nc.vector.tensor_scalar(
                                    out=s[:, j * 128:(j + 1) * 128],
                                    in0=iota_f[:],
                                    scalar1=ldst_s[:, pcw + j:pcw + j + 1],
                                    scalar2=wght_s[:, pcw + j:pcw + j + 1],
                                    op0=mybir.AluOpType.is_equal,
                                    op1=mybir.AluOpType.mult)
                        g = gpool.tile([128, bmax * es], gdt, tag=gtag)
                        gb = gpool.tile([128, pmax * F], BF16, tag="Gb")
                        if do_gather:
                            # true-count fetch: the tail slots of the last
                            # block keep whatever the buffer held (zeroed
                            # at startup, stale rows later - always
                            # finite); S has zero columns there.
                            ni_g = ni
                            nc.gpsimd.dma_gather(
                                out_ap=g[:, :nb * es].rearrange(
                                    "p (b f) -> p b f", b=nb),
                                in_ap=tp,
                                idxs_ap=idx_s[:, bb * 8:(bb + nb) * 8],
                                num_idxs=ni_g,
                                num_idxs_reg=ni_g,
                                elem_size=es,
                                single_packet=SINGLE_PACKET,
                                queue_num=gctr[0] % NQ,
                            )
                            gctr[0] += 1
                        if not do_compute:
                            continue
                        # upcast each sub-group's touched blocks to bf16
                        # (fused parity/quadrant select) on Activation;
                        # pair column k of sub h <-> local block tb0+k
                        g3 = g[:, :nb * es].rearrange("p (b f) -> p b f",
                                                      b=nb)
                        gb3 = gb[:, :npw * F].rearrange("p (c f) -> p c f",
                                                        c=npw)
                        co = 0
                        for (h, tb0, nt) in subs:
                            if do_gather:
                                nc.scalar.copy(
                                    out=gb3[:, co:co + nt, :],
                                    in_=g3[:, tb0:tb0 + nt,
                                           h * F:(h + 1) * F])
                            else:
                                # replay variant: source the iota const
                                # (same shape/dtype -> same Act timing)
                                io_ap = iota_f[:, 0:F]
                                nc.scalar.copy(
                                    out=gb3[:, co:co + nt, :],
                                    in_=bass.AP(io_ap.tensor, io_ap.offset,
                                                [io_ap.ap[0], [0, nt],
                                                 io_ap.ap[1]]))
                            co += nt
                        for j in range(npw):
                            nc.tensor.matmul(
                                out=pw[:],
                                lhsT=gb[:, j * F:(j + 1) * F],
                                rhs=s[:, j * 128:(j + 1) * 128],
                                start=(j == 0),
                                stop=(j == npw - 1))
                        nc.scalar.copy(out=dst[:, w * 128:(w + 1) * 128],
                                       in_=pw[:])

                tp1 = bass.AP(x_full.tensor, x_full.offset,
                              [[ES1, NP // NSUB1], [1, ES1]])
                tp2 = bass.AP(hp_full.tensor, hp_full.offset,
                              [[ES2, NP // NSUB2], [1, ES2]])

                txT = bpool.tile([F, SN], BF16, tag="txT")
                hT = bpool.tile([HID, SN], BF16, tag="hT")
                hp_sb = bpool.tile([128, W * F], DT2, tag="hp_sb")
                tx2T = bpool.tile([F, SN], BF16, tag="tx2T")
                out_sb = bpool.tile([128, W * OUT], FP32, tag="out_sb")
                nchunk = (SN + 511) // 512

                # ---- dense layer 1: hT = relu(W01.T @ xT + W11.T @ txT + b1)
                def dense1(c0, c1):
                    for lo in range(c0, c1, 512):
                        m = min(512, c1 - lo)
                        pd = psd.tile([HID, 512], FP32, tag="pd")
                        nc.tensor.matmul(out=pd[:, :m], lhsT=wts["w01"][:],
                                         rhs=xT[:, lo:lo + m], start=True,
                                         stop=False)
                        nc.tensor.matmul(out=pd[:, :m], lhsT=wts["w11"][:],
                                         rhs=txT[:, lo:lo + m], start=False,
                                         stop=True)
                        nc.scalar.activation(
                            out=hT[:, lo:lo + m], in_=pd[:, :m],
                            func=mybir.ActivationFunctionType.Relu,
                            bias=b1_s[:], scale=1.0)

                # ---- h stripe -> collective dtype (node-major)
                def hp_compute(w0, w1):
                    for w in range(w0, w1):
                        ps_h = pst.tile([128, F], BF16, tag="pt")
                        nc.tensor.transpose(out=ps_h[:],
                                            in_=hT[:, w * 128:(w + 1) * 128],
                                            identity=ident_b[:F, :F])
                        nc.vector.tensor_copy(
                            hp_sb[:, w * F:(w + 1) * F], ps_h[:])

                # ---- layer 1 + the h AllGather, pipelined per window
                # group: each group's collective slice fires as soon as its
                # windows' dense1 + cast are done, overlapping the rest of
                # layer-1 compute. hp_b row = W0*128 + p*ng + (wl-W0), the
                # concat order the per-group AllGather produces (= _r2).
                for (gw0, gw1) in WGRP:
                    ng = gw1 - gw0
                    matvec(tp1, ES1, NSUB1, FP32, "Ga", txT, idx_a,
                           ldst_a, wght_a, groups1, gw0, gw1,
                           do_gather=gather_on, do_compute=compute_on)
                    if compute_on:
                        dense1(gw0 * 128, gw1 * 128)
                        hp_compute(gw0, gw1)
                        nc.sync.dma_start(
                            out=hp_b[gw0 * 128:gw1 * 128].rearrange(
                                "(p j) f -> p j f", p=128),
                            in_=hp_sb[:, gw0 * F:gw1 * F].rearrange(
                                "p (j f) -> p j f", j=ng))
                    if coll_on:
                        nc.gpsimd.collective_compute(
                            "AllGather",
                            mybir.AluOpType.bypass,
                            ins=[hp_b[gw0 * 128:gw1 * 128]],
                            outs=[hp_full[gw0 * 128 * C:gw1 * 128 * C]],
                            replica_groups=rg,
                        )

                # ---- sparse matvec 2 (gathers straight from the AllGather
                # output - row order is _r2, no post-collective shuffle),
                # interleaved with dense layer 2 per 512-node chunk
                idx_b, ldst_b, wght_b = load_edges(idx16b, ldstb, wghtb,
                                                   nbtot2, npairs2)
                o_all = bpool.tile([128, W * OUT], FP32, tag="o_all")

                def dense2(i):
                    lo = i * 512
                    m = min(512, SN - lo)
                    pd = psd.tile([OUT, 512], FP32, tag="pd")
                    nc.tensor.matmul(out=pd[:, :m], lhsT=wts["w02"][:],
                                     rhs=hT[:, lo:lo + m], start=True,
                                     stop=False)
                    nc.tensor.matmul(out=pd[:, :m], lhsT=wts["w12"][:],
                                     rhs=tx2T[:, lo:lo + m], start=False,
                                     stop=True)
                    ob = wpool.tile([OUT, 512], FP32, tag="ob")
                    nc.vector.tensor_scalar(
                        out=ob[:, :m], in0=pd[:, :m], scalar1=b2_s[:],
                        scalar2=None, op0=mybir.AluOpType.add)
                    for j in range(m // 128):
                        w = i * 4 + j
                        ps_o = pst.tile([128, OUT], FP32, tag="pt")
                        nc.tensor.transpose(
                            out=ps_o[:], in_=ob[:, j * 128:(j + 1) * 128],
                            identity=ident[:OUT, :OUT])
                        nc.vector.tensor_copy(
                            o_all[:, w * OUT:(w + 1) * OUT], ps_o[:])

                for i in range(nchunk):
                    matvec(tp2, ES2, NSUB2, DT2, "Gl2", tx2T, idx_b,
                           ldst_b, wght_b, groups2, i * 4,
                           min((i + 1) * 4, W),
                           do_gather=gather_on, do_compute=compute_on)
                    if compute_on:
                        dense2(i)
                if not compute_on:
                    return

                def softmax_tail():
                    e_all = bpool.tile([128, W * OUT], FP32, tag="e_all")
                    nc.scalar.activation(out=e_all[:], in_=o_all[:],
                                         func=mybir.ActivationFunctionType.Exp)
                    ssum = wpool.tile([128, W], FP32, tag="ssum")
                    nc.vector.tensor_reduce(
                        out=ssum[:],
                        in_=e_all[:].rearrange("p (w q) -> p w q", w=W),
                        axis=AX.X, op=mybir.AluOpType.add)
                    lns = wpool.tile([128, W], FP32, tag="lns")
                    nc.scalar.activation(out=lns[:], in_=ssum[:],
                                         func=mybir.ActivationFunctionType.Ln)
                    ln_ap = lns[:]
                    nc.vector.tensor_tensor(
                        out=out_sb[:].rearrange("p (w q) -> p w q", w=W),
                        in0=o_all[:].rearrange("p (w q) -> p w q", w=W),
                        in1=bass.AP(ln_ap.tensor, ln_ap.offset,
                                    [ln_ap.ap[0], ln_ap.ap[1], [0, OUT]]),
                        op=mybir.AluOpType.subtract)

                softmax_tail()
                nc.sync.dma_start(
                    out=out.rearrange("(w p) f -> p w f", p=128),
                    in_=out_sb[:].rearrange("p (w f) -> p w f", w=W))

            rm = os.environ.get("CHEB_RM", "full")
            body("full")
            for _ in range(replay):
                body(rm)

    nc.compile()
    return nc


# ------------------------------------------------------------------- driver

_CACHE = {}


def _get_program_and_maps(x, edge_index, W0_1, W1_1, b1, W0_2, W1_2, b2):
    pk1, pk2 = _pack(np.asarray(edge_index))
    idx16a, ldesta, groups1, nbtot1, wghta, npairs1 = pk1
    idx16b, ldestb, groups2, nbtot2, wghtb, npairs2 = pk2

    x_pad = np.zeros((NP, F), dtype=np.float32)
    x_pad[:N] = np.asarray(x, dtype=np.float32)

    key = tuple(v for g in groups1 + groups2 for v in g)
    if key not in _CACHE:
        _CACHE[key] = _build(groups1, nbtot1, groups2, nbtot2)
    nc = _CACHE[key]

    shared = {
        "x_full": x_pad,
        "w01": np.asarray(W0_1, np.float32),
        "w11": np.asarray(W1_1, np.float32),
        "b1": np.asarray(b1, np.float32).reshape(HID, 1),
        "w02": np.asarray(W0_2, np.float32),
        "w12": np.asarray(W1_2, np.float32),
        "b2": np.asarray(b2, np.float32).reshape(OUT, 1),
    }
    in_maps = []
    for c in range(C):
        m = dict(shared)
        m["x_st"] = np.ascontiguousarray(x_pad[c * SN:(c + 1) * SN])
        m["idx16a"] = np.ascontiguousarray(idx16a[c])
        m["ldsta"] = np.ascontiguousarray(ldesta[c])
        m["wghta"] = np.ascontiguousarray(wghta[c])
        m["idx16b"] = np.ascontiguousarray(idx16b[c])
        m["ldstb"] = np.ascontiguousarray(ldestb[c])
        m["wghtb"] = np.ascontiguousarray(wghtb[c])
        in_maps.append(m)
    return nc, in_maps


def kernel(x, edge_index, W0_1, W1_1, b1, W0_2, W1_2, b2, **kw):
    nc, in_maps = _get_program_and_maps(
        x, edge_index, W0_1, W1_1, b1, W0_2, W1_2, b2)

    from concourse.bass_utils import run_bass_kernel_spmd

    res = run_bass_kernel_spmd(nc, in_maps, core_ids=list(range(C)))
    outs = [res.results[c]["out"] for c in range(C)]
    full = np.concatenate(outs, axis=0)[:N]
    return full.astype(np.float32)
